# revision 56
# baseline (speedup 1.0000x reference)
"""Causal self-attention block on 8 Trainium2 NeuronCores.

Reference computation (B=4, T=2048, D=1024, H=16, hd=64):
    qkv = x @ Wqkv + bqkv ; per-head causal softmax(q k^T / sqrt(hd)) v ;
    out = concat_heads @ Wproj + bproj

Sharding: core c = (batch b = c//2, head-group g = c%2 of 8 heads).
Each core computes its batch's qkv for its 8 heads, the causal attention,
and a partial projection (its 512 rows of Wproj). Host sums the two
head-group partials per batch and adds bproj.

All matmuls run bf16 x bf16 with fp32 PSUM accumulation. The three phases
are fused into one software pipeline per 512-wide t-chunk. Diagonal-block
scores/AV matmuls and the exp() are compacted to the causally-valid q
range; the remaining per-128-tile triangle is masked with a single
[128,2,128] vector multiply. q/k/v all stay resident in SBUF. Softmax
denominators come free from a ones-column in the AV stationary;
normalization uses a DRAM-bounce partition broadcast.

Schedule (from trace-driven tuning, 302.6us -> ~282us):
- Startup: x0 + the first head-pair-group's WQ/WK ride the two fast
  HWDGE DMA queues in fine (256KB) pieces ordered by first-use; only
  q/k of head-pairs 0/1 (+v) gate the first attention; q23/k23 and all
  later-chunk QKV run as paced fillers. WP loads at j=1, x1 from the
  scalar stream (an idle engine would issue it early and steal HBM
  bandwidth from the startup crunch).
- The attention phase alone is ACT(exp)-bound ~1.43x, so filler matmul
  work is allocated per chunk-section to cover the exp overhang; all
  proj work fills the last (largest) section.
- Chunk-3 proj is split: ko 0..2 accumulate into SBUF as late fillers;
  after the final norm only ko=3 matmuls + adds + fp16 stores remain.
- OUT is stored fp16 (host upconverts and sums partials): halves the
  tail store traffic; final error is unchanged (~4.1e-3 absmax-rel).
"""

import numpy as np

B, T, D, H, HD = 4, 2048, 1024, 16, 64
NCORES = 8
HPG = H // 2          # heads per group: 8
C = HPG * HD          # per-core q/k/v columns: 512
KO = D // 128         # 8 input-dim k-tiles
NCH = T // 512        # 4 t-chunks
NT = T // 128         # 16 t-tiles
CG = C // 128         # 4 col-groups per q/k
SCALE = 1.0 / np.sqrt(HD)

_CACHE = {}


def _build():
    import functools
    import concourse.mybir as mybir
    import concourse.tile as tile
    from concourse import bacc
    import concourse.bass as bass

    F32 = mybir.dt.float32
    BF16 = mybir.dt.bfloat16
    AF = mybir.ActivationFunctionType

    nc = bacc.Bacc("TRN2", target_bir_lowering=False, debug=False,
                   num_devices=NCORES)
    # XBF/WK/WV/WP are pre-swizzled host-side to partition-major layouts so
    # DMA partition lines are 4-8KB contiguous (full HBM throughput).
    XBF = nc.declare_dram_parameter("XBF", [128, NCH, KO, 512], BF16,
                                    isOutput=False)
    # WQ/WK are cg-pair-major: [partition, head-pair-group(2), ko, 256] so
    # the S0 startup phase (head-pairs 0/1 only) needs just half the bytes.
    WQ = nc.declare_dram_parameter("WQ", [128, 2, KO, 256], BF16,
                                   isOutput=False)
    WK = nc.declare_dram_parameter("WK", [128, 2, KO, 256], BF16,
                                   isOutput=False)
    WV = nc.declare_dram_parameter("WV", [128, KO, C], BF16, isOutput=False)
    BQ = nc.declare_dram_parameter("BQ", [C], F32, isOutput=False)
    BK = nc.declare_dram_parameter("BK", [C], F32, isOutput=False)
    BV = nc.declare_dram_parameter("BV", [C], F32, isOutput=False)
    WP = nc.declare_dram_parameter("WP", [128, CG, D], BF16, isOutput=False)
    MASKS = nc.declare_dram_parameter("MASKS", [128, 2, 128], BF16,
                                      isOutput=False)
    F16 = mybir.dt.float16
    OUT = nc.declare_dram_parameter("OUT", [T, D], F16, isOutput=True)

    xbf_r = XBF[:, :, :, :]
    wq_r = WQ[:, :, :, :]
    wk_r = WK[:, :, :, :]
    wv_r = WV[:, :, :]
    wp_r = WP[:, :, :]
    bq_r = BQ[:].rearrange("(cg p) -> p cg", p=128)
    bk_r = BK[:].rearrange("(cg p) -> p cg", p=128)
    bv_ap = BV[:]
    bv_bcast = bass.AP(tensor=bv_ap.tensor, offset=bv_ap.offset,
                       ap=[[0, 128]] + list(bv_ap.ap))

    from contextlib import ExitStack
    with tile.TileContext(nc) as tc:
        with ExitStack() as ctx:
            def pool(name, bufs, space="SBUF"):
                return ctx.enter_context(
                    tc.tile_pool(name=name, bufs=bufs, space=space))
            consts = pool("consts", 1)
            ktp = pool("ktp", 1)
            vtp = pool("vtp", 1)
            ytp = pool("ytp", 1)
            qtp = pool("qtp", 1)
            wp1 = pool("wp1", 1)
            xs = pool("xs", 3)
            pep = pool("pe", 4)
            rbp = pool("rb", 2)
            rcp = pool("rc", 2)
            ostp = pool("ost", 2)
            dnp = pool("dn", 3)
            dramd = pool("dramd", 4, space="DRAM")
            ps1 = pool("ps1", 2, space="PSUM")
            ps_p = pool("psp", 2, space="PSUM")
            ps_oa = pool("psa", 2, space="PSUM")

            WQ_sb = wp1.tile([128, 2, KO, 256], BF16)
            WK_sb = wp1.tile([128, 2, KO, 256], BF16)
            WV_sb = wp1.tile([128, KO, C], BF16)
            WP_sb = wp1.tile([128, CG, D], BF16)

            masks_sb = consts.tile([128, 2, 128], BF16)
            bq_sb = consts.tile([128, CG], F32)
            bk_sb = consts.tile([128, CG], F32)
            bv_sb = consts.tile([128, C], F32)
            zt_sb = consts.tile([128, 512], BF16)
            ones_sb = consts.tile([128, 64], F32)

            kT_sb = ktp.tile([128, CG, T], BF16)      # [kcol%128, cg, t]
            # v columns 0:64 per (t-tile, head) plus ONE ones-column (65)
            # feeding the softmax denominator row of the AV output.
            VW = HD + 1
            v_aug = vtp.tile([128, NT, HPG, VW], BF16)
            yT_sb = ytp.tile([128, CG, T], BF16)      # [ycol%128, cg, t]
            qT_sb = qtp.tile([128, CG, T], BF16)      # [qcol%128, cg, t]

            # ---- startup DMAs: ordered + spread across the engine-owned
            # DMA queue groups so the S0 phase's inputs (x0, WQ/WK gpair-0,
            # WV) land first and late-needed tensors queue BEHIND them.
            # gpsimd(SWDGE): x chunk-0 per-2ko;
            # sync(HWDGE): WQ gp0, WV halves, WQ gp1;
            # scalar(HWDGE): biases + masks (tiny), WK gp0, bv, WK gp1.
            # WP is first needed by the proj fillers in the j=3 section
            # (~200us); its load is issued at the start of j=1, out of the
            # startup HBM crunch.
            # x0 rides the two HWDGE queues (sync half / scalar half): the
            # gpsimd SWDGE path moves bulk data several times slower and
            # was the S0 pacing laggard. Ordered by first-use time.
            # zt feeds the warmup dummies: memset it on gpsimd (idle at
            # start) so the dummies aren't gated on the vector engine's
            # slower init
            # Queue balance: ALL of x0 paces q01 from sync in per-2ko
            # pieces (never stalling behind weight blocks); the first-
            # needed weights stream from scalar in 256KB pieces; WV is
            # split across both queues and the S0 v-steps run ko-major so
            # each WV piece is needed as late as possible.
            nc.gpsimd.memset(zt_sb[:], 0.0)
            nc.vector.memset(ones_sb[:], 1.0)
            xc0 = xs.tile([128, KO, 512], BF16, tag="xc", name="xc")
            # the leading x0/WQ pieces are halved again (128KB): the very
            # first matmul is gated on their first-byte-to-last-byte
            # latency, everything later is pipelined anyway
            nc.sync.dma_start(out=xc0[:, 0:1, :], in_=xbf_r[:, 0, 0:1, :])
            nc.sync.dma_start(out=xc0[:, 1:2, :], in_=xbf_r[:, 0, 1:2, :])
            nc.sync.dma_start(out=xc0[:, 2:4, :], in_=xbf_r[:, 0, 2:4, :])
            nc.sync.dma_start(out=xc0[:, 4:6, :], in_=xbf_r[:, 0, 4:6, :])
            nc.sync.dma_start(out=xc0[:, 6:8, :], in_=xbf_r[:, 0, 6:8, :])
            nc.sync.dma_start(out=WV_sb[:, 0:4, :], in_=wv_r[:, 0:4, :])
            nc.sync.dma_start(out=WQ_sb[:, 1, :, :], in_=wq_r[:, 1, :, :])
            nc.scalar.dma_start(out=WQ_sb[:, 0, 0:2, :],
                                in_=wq_r[:, 0, 0:2, :])
            nc.scalar.dma_start(out=WQ_sb[:, 0, 2:4, :],
                                in_=wq_r[:, 0, 2:4, :])
            nc.scalar.dma_start(out=WQ_sb[:, 0, 4:8, :],
                                in_=wq_r[:, 0, 4:8, :])
            nc.scalar.dma_start(out=WK_sb[:, 0, 0:4, :],
                                in_=wk_r[:, 0, 0:4, :])
            nc.scalar.dma_start(out=WK_sb[:, 0, 4:8, :],
                                in_=wk_r[:, 0, 4:8, :])
            nc.scalar.dma_start(out=bq_sb[:], in_=bq_r)
            nc.scalar.dma_start(out=bk_sb[:], in_=bk_r)
            nc.scalar.dma_start(out=masks_sb[:], in_=MASKS[:, :, :])
            nc.scalar.dma_start(out=bv_sb[:], in_=bv_bcast)
            nc.scalar.dma_start(out=WV_sb[:, 4:8, :], in_=wv_r[:, 4:8, :])
            nc.scalar.dma_start(out=WK_sb[:, 1, :, :], in_=wk_r[:, 1, :, :])

            nc.vector.memset(v_aug[:, :, :, HD:VW], 1.0)

            # PE warmup / HAM keep-alive: dummy matmuls with no data deps.
            # Wide ones warm the clock before real data lands; narrow (64)
            # ones interleave with the DMA-paced S0 steps so a stall can
            # never open a >3.4us PE-idle window (which would re-throttle
            # the PE clock to 1.2 GHz). They accumulate into ps_p, which is
            # idle until attention starts.
            def dummy_mm(w=512):
                psd = ps_p.tile([128, 1024], F32, tag="pps", name="warm")
                nc.tensor.matmul(psd[:, 0:w], zt_sb[:, 0:128], zt_sb[:, 0:w],
                                 start=True, stop=True)
            # just enough cold dummies (~2.6us) to bridge until the first
            # x0/WQ pieces land; continuous PE-busy (dummy or real) warms
            # the HAM clock either way, and extra dummies would delay the
            # first data-ready matmul behind them in the in-order queue
            for _ in range(6):
                dummy_mm()

            # ---- phase-1 step closures for one t-chunk (run as fillers) ---
            def ph1_steps(tci, box=None, split0=False):
                tsl = slice(512 * tci, 512 * (tci + 1))
                if box is None:
                    box = {}

                def load_x():
                    if "xc" in box:
                        return
                    box["xc"] = xs.tile([128, KO, 512], BF16, tag="xc",
                                        name="xc")
                    # chunk 1 rides the sync queue: its packets drain
                    # BEHIND the startup-critical loads (FIFO ring), so it
                    # lands ~33us — in time for the first chunk-1 fillers —
                    # without stealing early HBM bandwidth. x2/x3 stay on
                    # gpsimd (gated behind the norm bounce DMAs); putting
                    # them on sync stalls that engine on the xs-pool
                    # rotation (measured worse).
                    eng = nc.sync if tci == 1 else nc.gpsimd
                    eng.dma_start(out=box["xc"][:],
                                  in_=xbf_r[:, tci, :, :])

                def qk_step(which, pair, ko, st_box, pool, ptag):
                    W_sb = WQ_sb if which == "q" else WK_sb
                    b_sb = bq_sb if which == "q" else bk_sb
                    gp = pair[0] // 2
                    if ko == 0:
                        st_box["ps"] = [
                            pool.tile([128, 512], F32, tag=ptag,
                                      name=f"p{which}{cg}")
                            for cg in pair]
                    for idx, cg in enumerate(pair):
                        sub = 128 * (cg % 2)
                        nc.tensor.matmul(
                            st_box["ps"][idx][:],
                            W_sb[:, gp, ko, sub:sub + 128],
                            box["xc"][:, ko, :],
                            start=(ko == 0), stop=(ko == KO - 1))
                    if ko == KO - 1:
                        dst = qT_sb if which == "q" else kT_sb
                        for idx, cg in enumerate(pair):
                            nc.vector.tensor_scalar_add(
                                out=dst[:, cg, tsl],
                                in0=st_box["ps"][idx][:],
                                scalar1=b_sb[:, cg:cg + 1])

                def v_step(pair, ko, st_box, pool, ptag):
                    if ko == 0:
                        st_box["ps"] = [
                            pool.tile([128, 512], F32, tag=ptag,
                                      name=f"pv{tt}")
                            for tt in pair]
                    for idx, tt in enumerate(pair):
                        nc.tensor.matmul(
                            st_box["ps"][idx][:],
                            box["xc"][:, ko, 128 * tt:128 * (tt + 1)],
                            WV_sb[:, ko, :],
                            start=(ko == 0), stop=(ko == KO - 1))
                    if ko == KO - 1:
                        for idx, tt in enumerate(pair):
                            kj = 4 * tci + tt
                            nc.vector.tensor_add(
                                v_aug[:, kj, :, 0:HD],
                                st_box["ps"][idx][:].rearrange(
                                    "p (h d) -> p h d", d=HD),
                                bv_sb[:, :].rearrange(
                                    "p (h d) -> p h d", d=HD))

                def plist(fn, *args, pairs, pools):
                    out = []
                    for pair, (pool, ptag) in zip(pairs, pools):
                        sb = {}
                        for ko in range(KO):
                            out.append(functools.partial(
                                fn, *args, pair, ko, sb, pool, ptag))
                    return out

                P1 = (ps1, "ph1")
                POA = (ps_oa, "oaug")
                if split0:
                    # S0: only head-pairs 0/1 of q/k (plus all of v) gate
                    # the start of attention; q23/k23 run as j=0 fillers.
                    # k01/v23 borrow the (idle until attention) ps_oa bufs
                    # so the phases don't serialize through ps1's 2 banks.
                    v01 = plist(v_step, pairs=((0, 1),), pools=(P1,))
                    v23 = plist(v_step, pairs=((2, 3),), pools=(POA,))
                    # ko-major v order: each WV[ko] piece is consumed as
                    # late as possible (tracks the split WV DMA arrival)
                    vko = [s for two in zip(v01, v23) for s in two]
                    s0 = (plist(qk_step, "q", pairs=((0, 1),), pools=(P1,))
                          + plist(qk_step, "k", pairs=((0, 1),),
                                  pools=(POA,))
                          + vko)
                    late = (plist(qk_step, "q", pairs=((2, 3),),
                                  pools=(P1,))
                            + plist(qk_step, "k", pairs=((2, 3),),
                                    pools=(P1,)))
                    return s0, late
                qsteps = ([load_x] if tci > 0 else []) + plist(
                    qk_step, "q", pairs=((0, 1), (2, 3)), pools=(P1, P1))
                kvsteps = (plist(qk_step, "k", pairs=((0, 1), (2, 3)),
                                 pools=(P1, P1))
                           + plist(v_step, pairs=((0, 1), (2, 3)),
                                   pools=(P1, P1)))
                return qsteps, kvsteps

            # ---- proj step closures for one t-chunk ----
            def proj_steps(jj):
                steps = []

                def do(tt, n, st_box):
                    if n == 0:
                        st_box["ost"] = ostp.tile([128, D], F16, tag="ost",
                                                  name="ost")
                    po = ps1.tile([128, 512], F32, tag="ph1", name="po")
                    for ko in range(CG):
                        nc.tensor.matmul(
                            po[:],
                            yT_sb[:, ko, 128 * tt:128 * (tt + 1)],
                            WP_sb[:, ko, 512 * n:512 * (n + 1)],
                            start=(ko == 0), stop=(ko == CG - 1))
                    nc.vector.tensor_copy(
                        out=st_box["ost"][:, 512 * n:512 * (n + 1)],
                        in_=po[:])
                    nc.sync.dma_start(
                        out=OUT[128 * tt:128 * (tt + 1),
                                512 * n:512 * (n + 1)],
                        in_=st_box["ost"][:, 512 * n:512 * (n + 1)])

                for tt in range(4 * jj, 4 * jj + 4):
                    sb = {}
                    for n in range(2):
                        steps.append(functools.partial(do, tt, n, sb))
                return steps

            # ---- chunk-3 proj is split: ko 0..2 accumulate into SBUF as
            # late j=3 fillers (they only need norms i=0..2); after the
            # final norm only the ko=3 matmul + add + store remain.
            acc3 = wp1.tile([128, 8, 512], F32)
            P3U = [(tt, n) for tt in range(12, 16) for n in range(2)]

            def proj3_partial_steps():
                def pa(u_i, tt, n):
                    po = ps1.tile([128, 512], F32, tag="ph1", name="po3a")
                    for ko in range(CG - 1):
                        nc.tensor.matmul(
                            po[:],
                            yT_sb[:, ko, 128 * tt:128 * (tt + 1)],
                            WP_sb[:, ko, 512 * n:512 * (n + 1)],
                            start=(ko == 0), stop=(ko == CG - 2))
                    nc.vector.tensor_copy(out=acc3[:, u_i, :], in_=po[:])
                return [functools.partial(pa, u_i, tt, n)
                        for u_i, (tt, n) in enumerate(P3U)]

            def proj3_finish():
                ost = None
                for u_i, (tt, n) in enumerate(P3U):
                    if n == 0:
                        ost = ostp.tile([128, D], F16, tag="ost", name="ost")
                    po = ps1.tile([128, 512], F32, tag="ph1", name="po3b")
                    nc.tensor.matmul(
                        po[:],
                        yT_sb[:, CG - 1, 128 * tt:128 * (tt + 1)],
                        WP_sb[:, CG - 1, 512 * n:512 * (n + 1)],
                        start=True, stop=True)
                    # (gpsimd cannot read PSUM, so the adds stay on DVE)
                    nc.vector.tensor_add(
                        ost[:, 512 * n:512 * (n + 1)], po[:],
                        acc3[:, u_i, :])
                    nc.sync.dma_start(
                        out=OUT[128 * tt:128 * (tt + 1),
                                512 * n:512 * (n + 1)],
                        in_=ost[:, 512 * n:512 * (n + 1)])

            # ---- S0: the minimal chunk-0 QKV subset gating attention runs
            # up front (keep-alive dummies between steps are useless: the
            # PE queue is in-order, so a dummy behind a stalled matmul
            # cannot fill its wait)
            s0, late0 = ph1_steps(0, box={"xc": xc0}, split0=True)
            for s in s0:
                s()

            # ---- fused attention + fillers ----
            def ph1_all(tci):
                q, kv = ph1_steps(tci)
                return q + kv

            def j1_steps():
                # deferred WP load (needed by the proj fillers at j=3)
                nc.scalar.dma_start(out=WP_sb[:, 0:2, :], in_=wp_r[:, 0:2, :])
                nc.scalar.dma_start(out=WP_sb[:, 2:4, :], in_=wp_r[:, 2:4, :])
                return ph1_all(2)

            # The attention phase alone is ACT(exp)-bound by ~1.43x, so each
            # j-section needs PE filler work >= 0.43x its attention matmul
            # time or the PE idles behind the exp stream. The proj fillers
            # (20.4us) therefore all go to j=3 (overhang ~21us).
            filler_plan = {
                0: lambda: late0 + ph1_all(1),
                1: j1_steps,
                2: lambda: ph1_all(3),
                3: lambda: (proj_steps(0) + proj_steps(1) + proj_steps(2)
                            + proj3_partial_steps()),
            }
            RESERVE = 5     # filler steps held back to cover the final
            fillers = []    # norm-chain latency before proj3_finish
                            # (5 measured best; 3 starved the drain)
            CARRY = 2       # steps held across each j-boundary; they run
            carried = []    # inline before the next section's first scores
            for j in range(NCH):
                for s in carried:
                    s()
                carried = []
                fillers.extend(filler_plan[j]())
                npairs = 2 * j + 2
                nkj = 4 * j + 4
                total_av_steps = CG * npairs
                hold = RESERVE if j == NCH - 1 else CARRY
                nfill = max(0, len(fillers) - hold)
                done_av = 0
                popped = 0

                # per-half causal compaction: k-tile m vs q-chunk j
                def geom(u, half, j=j):
                    m = 2 * u + half
                    mloc = m - 4 * j
                    qoff = 128 * mloc if mloc >= 0 else 0
                    return m, qoff, 512 - qoff

                for i in range(CG):
                    oaug = [ps_oa.tile([128, 512], F32, tag="oaug",
                                       name=f"oaug{hh}")
                            for hh in range(2)]
                    pexp = {}

                    def emit_scores(u, i=i, j=j, pexp=pexp):
                        diag = (2 * u >= 4 * j)
                        for hh in range(2):
                            base = 64 * hh
                            pps = ps_p.tile([128, 1024], F32, tag="pps",
                                            name=f"pps{hh}")
                            for half in range(2):
                                m, qoff, w = geom(u, half)
                                nc.tensor.matmul(
                                    pps[:, 512 * half:512 * half + w],
                                    kT_sb[base:base + 64, i,
                                          128 * m:128 * (m + 1)],
                                    qT_sb[base:base + 64, i,
                                          512 * j + qoff:512 * j + qoff + w],
                                    start=True, stop=True)
                            pe = pep.tile([128, 1024], BF16, tag="pe",
                                          name=f"pe{hh}")
                            if not diag:
                                nc.scalar.activation(
                                    out=pe[:], in_=pps[:], func=AF.Exp,
                                    scale=float(SCALE))
                            elif 2 * u == 4 * j:
                                # halves are [0,512) and [512,896):
                                # contiguous -> one ACT over [0,896)
                                nc.scalar.activation(
                                    out=pe[:, 0:896], in_=pps[:, 0:896],
                                    func=AF.Exp, scale=float(SCALE))
                            else:
                                # halves are [0,256) and [512,640): one ACT
                                # over [0,640) — the [256,512) gap is stale
                                # PSUM whose exp lands in pe cols never read
                                # by the AV matmuls (saves one ACT dispatch)
                                nc.scalar.activation(
                                    out=pe[:, 0:640], in_=pps[:, 0:640],
                                    func=AF.Exp, scale=float(SCALE))
                            if diag:
                                # mask the leading 128-col band of each half
                                pe3 = pe[:].rearrange("p (r c) -> p r c",
                                                      c=512)
                                nc.vector.tensor_mul(
                                    pe3[:, :, 0:128], pe3[:, :, 0:128],
                                    masks_sb[:, :, :])
                            pexp[(hh, u)] = pe

                    emit_scores(0)
                    for u in range(npairs):
                        if u + 1 < npairs:
                            emit_scores(u + 1)
                        for hh in range(2):
                            h = 2 * i + hh
                            pe = pexp.pop((hh, u))
                            for half in range(2):
                                m, qoff, w = geom(u, half)
                                nc.tensor.matmul(
                                    oaug[hh][0:VW, qoff:qoff + w],
                                    v_aug[:, m, h, :],
                                    pe[:, 512 * half:512 * half + w],
                                    start=(m == 0), stop=(m == nkj - 1))
                        done_av += 1
                        target = (nfill * done_av) // total_av_steps
                        while popped < target:
                            fillers[popped]()
                            popped += 1
                    # normalization, entirely off the PE/ACT engines;
                    # oaug is released by the first copy so the next
                    # head-pair's AV can start immediately
                    tail_i = (j == NCH - 1 and i == CG - 1)
                    if tail_i:
                        bc = ps_p.tile([128, 1024], F32, tag="pps",
                                       name="bc")
                    for hh in range(2):
                        base = 64 * hh
                        oc = dnp.tile([HD + 1, 512], F32, tag="dn")
                        nc.vector.tensor_copy(out=oc[:],
                                              in_=oaug[hh][0:HD + 1, :])
                        if tail_i:
                            # latency-critical last norm: broadcast the
                            # denominator row across partitions with a
                            # K=1 matmul instead of the DRAM bounce
                            nc.tensor.matmul(
                                bc[0:64, 512 * hh:512 * hh + 512],
                                ones_sb[64:65, :], oc[HD:HD + 1, :],
                                start=True, stop=True)
                            rb_ap = bc[0:64, 512 * hh:512 * hh + 512]
                        else:
                            dnd = dramd.tile([1, 512], F32, tag="dnd")
                            nc.gpsimd.dma_start(out=dnd[:, :],
                                                in_=oc[HD:HD + 1, :])
                            rb = rbp.tile([64, 512], F32, tag="rb")
                            dnd_ap = dnd[:, :]
                            nc.gpsimd.dma_start(
                                out=rb[:],
                                in_=bass.AP(tensor=dnd_ap.tensor,
                                            offset=dnd_ap.offset,
                                            ap=[[0, 64], [1, 512]]))
                            rb_ap = rb[:]
                        rc = rcp.tile([64, 512], F32, tag="rc")
                        nc.vector.reciprocal_approx_fast(out=rc[:],
                                                         in_=rb_ap)
                        nc.vector.tensor_mul(
                            yT_sb[base:base + 64, i,
                                  512 * j:512 * (j + 1)],
                            oc[0:HD, :], rc[:])
                keep = 0 if j == NCH - 1 else CARRY
                while popped < max(0, len(fillers) - keep):
                    fillers[popped]()
                    popped += 1
                carried = fillers[popped:]
                del fillers[:]

            proj3_finish()

    nc.compile()
    return nc


def _masks():
    m = np.zeros((128, 2, 128), dtype=np.float32)
    cols = np.arange(128)
    for p in range(128):
        m[p, :, :] = cols >= p
    return m


def _prep_inputs(x, Wqkv, bqkv, Wproj, bproj):
    import ml_dtypes
    bf16 = ml_dtypes.bfloat16
    x = np.asarray(x, dtype=np.float32)
    Wqkv = np.asarray(Wqkv, dtype=np.float32)
    bqkv = np.asarray(bqkv, dtype=np.float32)
    Wproj = np.asarray(Wproj, dtype=np.float32)
    masks = _masks()

    def pmajor(W):
        # [R, C] -> [128, R//128, C]: partition-major for long DMA lines
        r, c = W.shape
        return np.ascontiguousarray(
            W.reshape(r // 128, 128, c).transpose(1, 0, 2))

    def cgmajor(W):
        # [D, 512] -> [128, 2, KO, 256]: cg-pair-major so the two head-pair
        # groups are contiguous DMA blocks (S0 loads only group 0)
        return np.ascontiguousarray(
            W.reshape(KO, 128, 2, 256).transpose(1, 2, 0, 3))

    in_maps = []
    for c in range(NCORES):
        b, g = c // 2, c % 2
        sl = slice(C * g, C * (g + 1))
        # x[b].T is [D, T]; swizzle to [128, NCH, KO, 512]
        xT = x[b].T.reshape(KO, 128, NCH, 512).transpose(1, 2, 0, 3)
        in_maps.append({
            "XBF": np.ascontiguousarray(xT).astype(bf16),
            "WQ": cgmajor(Wqkv[:, sl]).astype(bf16),
            "WK": cgmajor(Wqkv[:, D + C * g:D + C * (g + 1)]).astype(bf16),
            "WV": pmajor(
                Wqkv[:, 2 * D + C * g:2 * D + C * (g + 1)]).astype(bf16),
            "BQ": np.ascontiguousarray(bqkv[sl]),
            "BK": np.ascontiguousarray(bqkv[D + C * g:D + C * (g + 1)]),
            "BV": np.ascontiguousarray(bqkv[2 * D + C * g:2 * D + C * (g + 1)]),
            "WP": pmajor(Wproj[sl, :]).astype(bf16),
            "MASKS": masks.astype(bf16),
        })
    return in_maps


def _run(inputs, **run_kwargs):
    from concourse.bass_utils import run_bass_kernel_spmd
    if "nc" not in _CACHE:
        _CACHE["nc"] = _build()
    nc = _CACHE["nc"]
    in_maps = _prep_inputs(**inputs)
    res = run_bass_kernel_spmd(nc, in_maps, core_ids=list(range(NCORES)),
                               **run_kwargs)
    bproj = np.asarray(inputs["bproj"], dtype=np.float32)
    out = np.empty((B, T, D), dtype=np.float32)
    for b in range(B):
        # device partials are fp16; combine in fp32 on host
        out[b] = (res.results[2 * b]["OUT"].astype(np.float32)
                  + res.results[2 * b + 1]["OUT"].astype(np.float32))
        out[b] += bproj
    return out, res


def kernel(x, Wqkv, bqkv, Wproj, bproj):
    out, _ = _run(dict(x=x, Wqkv=Wqkv, bqkv=bqkv, Wproj=Wproj, bproj=bproj))
    return out

